# revision 12
# baseline (speedup 1.0000x reference)
"""Trainium2 Bass kernel for nn_DualSignalLinkPredictorC (2-layer GATv2 + MLP
link predictor), distributed over 8 NeuronCores.

v2: degree-sorted CSR layout. The backend executes ~1 instruction per 45us
regardless of width, so the design minimizes instruction count:
  - dst nodes on partitions, neighbors padded along the free dim (K_t = max
    in-degree per 128-node tile). Per-core nodes are permuted by descending
    degree so K_t tracks the local mean (~18) instead of the global max.
  - per-edge attention, segment softmax, and aggregation are wide DVE ops
    over [128, K_t*D] tiles; no one-hot matmuls, no PE transposes in the
    edge phase. Gathers are one indirect DMA per neighbor column.
  - single AllGather per table (no chunking; int32 row indices).
  - x is uploaded in fp8 (e4m3) to halve the dominant input transfer.
"""

import numpy as np
import ml_dtypes

BF16 = ml_dtypes.bfloat16
FP8 = ml_dtypes.float8_e4m3fn


class Cfg:
    def __init__(self, N=100000, E=1600000, NPAIRS=262144, NC=8,
                 RAW=512, IN=256, HID=256, EMB=128):
        self.N, self.E, self.NPAIRS, self.NC = N, E, NPAIRS, NC
        self.RAW, self.IN, self.HID, self.EMB = RAW, IN, HID, EMB
        assert N % NC == 0
        self.SH = N // NC
        self.T = (self.SH + 127) // 128
        self.PPC = NPAIRS // NC
        assert self.PPC % 128 == 0


CFG = Cfg()


class Plan:
    """Degree-sorted CSR neighbor plan + decode indices, per core."""

    def __init__(self, cfg, x, edge_index, edge_pairs):
        NC, SH, T = cfg.NC, cfg.SH, cfg.T
        ei = np.asarray(edge_index, dtype=np.int64)
        ep = np.asarray(edge_pairs, dtype=np.int64)
        loops = np.arange(cfg.N, dtype=np.int64)
        src = np.concatenate([ei[0], loops])
        dst = np.concatenate([ei[1], loops])

        deg = np.bincount(dst, minlength=cfg.N)
        # per-core descending-degree permutation; global_row maps node id ->
        # row in the AllGathered tables (core-major, rank within core).
        self.perm = []           # per core: rank -> local node
        global_row = np.empty(cfg.N, dtype=np.int64)
        for c in range(NC):
            dloc = deg[c * SH:(c + 1) * SH]
            p = np.argsort(-dloc, kind="stable")
            self.perm.append(p)
            rank = np.empty(SH, dtype=np.int64)
            rank[p] = np.arange(SH)
            global_row[c * SH:(c + 1) * SH] = c * SH + rank
        self.global_row = global_row

        srcrow = global_row[src]
        core_of = dst // SH
        rank_of = global_row[dst] - core_of * SH

        self.IDXE, self.DEGT, self.K_t, self.OFF_t = [], [], [], []
        for c in range(NC):
            m = core_of == c
            r = rank_of[m]
            v = srcrow[m]
            order = np.argsort(r, kind="stable")
            r, v = r[order], v[order]
            dsorted = np.zeros(SH, dtype=np.int64)
            dsorted[:SH] = np.bincount(r, minlength=SH)
            starts = np.concatenate([[0], np.cumsum(dsorted)])
            K_t = np.zeros(T, dtype=np.int64)
            for t in range(T):
                K_t[t] = dsorted[128 * t:128 * (t + 1)].max()
            OFF = np.concatenate([[0], np.cumsum(K_t)]).astype(np.int64)
            CK = int(OFF[-1])
            idxe = np.zeros((128, CK), dtype=np.int32)
            slot_in_dst = np.arange(len(r)) - starts[r]
            tile_of = r >> 7
            p_of = r & 127
            col = OFF[tile_of] + slot_in_dst
            idxe[p_of, col] = v
            degt = np.zeros((128, T), dtype=np.float32)
            dpad = np.concatenate([dsorted, np.zeros(T * 128 - SH, np.int64)])
            degt[:, :] = dpad.reshape(T, 128).T
            self.IDXE.append(np.ascontiguousarray(idxe))
            self.DEGT.append(degt)
            self.K_t.append(K_t)
            self.OFF_t.append(OFF)
        self.CKmax = max(int(o[-1]) for o in self.OFF_t)
        self.KMAX = max(int(k.max()) for k in self.K_t)
        # pad every core's IDXE to CKmax columns so shapes match SPMD
        for c in range(NC):
            ck = self.IDXE[c].shape[1]
            if ck < self.CKmax:
                self.IDXE[c] = np.ascontiguousarray(np.pad(
                    self.IDXE[c], ((0, 0), (0, self.CKmax - ck))))

        # decode: pair i of core c -> (p=i%128, col=i//128)
        self.PS, self.PD = [], []
        pr = global_row[ep[:, 0]].reshape(NC, cfg.PPC)
        qr = global_row[ep[:, 1]].reshape(NC, cfg.PPC)
        for c in range(NC):
            ps = pr[c].reshape(cfg.PPC // 128, 128).T.astype(np.int32)
            pd = qr[c].reshape(cfg.PPC // 128, 128).T.astype(np.int32)
            self.PS.append(np.ascontiguousarray(ps))
            self.PD.append(np.ascontiguousarray(pd))

        x = np.nan_to_num(np.asarray(x, dtype=np.float32), nan=0.0,
                          posinf=0.0, neginf=0.0)
        self.xT = []
        for c in range(NC):
            xs = x[c * SH:(c + 1) * SH][self.perm[c]]
            self.xT.append(np.ascontiguousarray(xs.T.astype(FP8)))


def host_prep(x, edge_index, edge_pairs, cfg):
    return Plan(cfg, x, edge_index, edge_pairs)


def prep_weights(inp, cfg):
    f = lambda a: np.asarray(a, np.float32)
    W = {}
    # projection weights ship sharded (1/8 per core) and are reassembled on
    # device by one AllGather; blob order must match build_program's offsets.
    blob = np.concatenate([
        f(inp["Wp"]).T.astype(BF16).ravel()] + [
        f(inp[k]).T.astype(BF16).ravel()
        for k in ("Wl1", "Wr1", "Wm1", "Wm2", "Wl2", "Wr2")])
    assert blob.size == 425984
    W["WBLOB"] = [np.ascontiguousarray(
        blob[c * 53248:(c + 1) * 53248].reshape(208, 256))
        for c in range(cfg.NC)]
    W["ATT1R"] = np.ascontiguousarray(np.broadcast_to(
        f(inp["att1"]).reshape(1, -1), (128, cfg.HID))).astype(BF16)
    W["ATT2R"] = np.ascontiguousarray(np.broadcast_to(
        f(inp["att2"]).reshape(1, -1), (128, cfg.EMB))).astype(BF16)
    W["IDENT"] = np.ascontiguousarray(np.eye(128, dtype=np.float32).astype(BF16))
    W["IOTA_ROWS"] = np.ascontiguousarray(np.broadcast_to(
        np.arange(128, dtype=np.float32), (128, 128))).copy()
    alpha = 1.0 / (1.0 + np.exp(-float(f(inp["logit_alpha"]).ravel()[0])))
    temp = float(f(inp["temperature"]))
    W["A12R"] = np.ascontiguousarray(np.broadcast_to(
        np.array([alpha * temp, (1.0 - alpha) * temp], np.float32),
        (128, 2))).copy()
    return W


# ---------------------------------------------------------------------------
# device program
# ---------------------------------------------------------------------------

def build_program(plan, cfg, stage=5):
    import contextlib
    import concourse.bass as bass
    import concourse.tile as tile
    from concourse import bacc, mybir

    dt = mybir.dt
    AF = mybir.ActivationFunctionType
    OP = mybir.AluOpType
    AX = mybir.AxisListType

    NC, SH, T = cfg.NC, cfg.SH, cfg.T
    RAW, IN, HID, EMB = cfg.RAW, cfg.IN, cfg.HID, cfg.EMB
    KQ = RAW // 128
    N8 = SH * NC
    CK = plan.CKmax
    KMAX = plan.KMAX
    K_t, OFF_t = plan.K_t[0], plan.OFF_t[0]   # identical structure per core?
    EPS_LN = 1e-5
    EPS_DEN = 1e-16

    # NOTE: K_t / OFF_t differ per core. SPMD emits ONE program, so loop
    # bounds must be core-independent: use the per-tile MAX over cores.
    K_t = np.stack([plan.K_t[c] for c in range(NC)]).max(axis=0)
    OFF = np.concatenate([[0], np.cumsum(K_t)]).astype(np.int64)
    assert OFF[-1] <= CK or True
    CKU = int(OFF[-1])

    nc = bacc.Bacc("TRN2", target_bir_lowering=False, debug=False,
                   num_devices=NC)

    din = lambda name, shape, d: nc.dram_tensor(name, shape, d,
                                                kind="ExternalInput").ap()
    xT = din("xT", [RAW, SH], dt.float8e4)
    IDXE = din("IDXE", [128, CKU], dt.int32)
    DEGT = din("DEGT", [128, T], dt.float32)
    PS32 = din("PS32", [128, cfg.PPC // 128], dt.int32)
    PD32 = din("PD32", [128, cfg.PPC // 128], dt.int32)
    WBLOB = din("WBLOB", [208, 256], dt.bfloat16)
    ATT1R = din("ATT1R", [128, HID], dt.bfloat16)
    ATT2R = din("ATT2R", [128, EMB], dt.bfloat16)
    IDENT = din("IDENT", [128, 128], dt.bfloat16)
    IOTA_ROWS = din("IOTA_ROWS", [128, 128], dt.float32)
    A12R = din("A12R", [128, 2], dt.float32)

    res_out = nc.dram_tensor("res", [cfg.PPC], dt.float32,
                             kind="ExternalOutput").ap()

    rg = [list(range(NC))]
    sems = {ph: nc.alloc_semaphore(f"gsem_{ph}") for ph in ("a", "b", "d")}
    gcnt = {ph: 0 for ph in ("a", "b", "d")}

    def rows(t):
        return min(128, SH - 128 * t)

    with tile.TileContext(nc) as tc:
        ctx = contextlib.ExitStack()
        with ctx:
            cpool = ctx.enter_context(tc.tile_pool(name="consts", bufs=1))
            dpool = ctx.enter_context(tc.tile_pool(name="dram", bufs=1,
                                                   space="DRAM"))
            sstat = ctx.enter_context(tc.tile_pool(name="sstat", bufs=2))
            dps = ctx.enter_context(tc.tile_pool(name="dps", bufs=2,
                                                 space="PSUM"))

            def cload(ap, shape, d=dt.bfloat16, name=None):
                t_ = cpool.tile(shape, d, name=name)
                nc.sync.dma_start(t_[:], ap)
                return t_

            # reassemble the sharded weight blob: upload -> own DRAM slice ->
            # AllGather -> per-weight strided loads into SBUF.
            wblob_own = dpool.tile([208, 256], dt.bfloat16, name="wblob_own")
            wblob = dpool.tile([1664, 256], dt.bfloat16, name="wblob",
                               addr_space="Shared")
            wsb = cpool.tile([128, 416], dt.bfloat16, name="wsb")
            nc.sync.dma_start(
                wsb[:], bass.AP(WBLOB.tensor, 0, [[416, 128], [1, 416]]))
            nc.sync.dma_start(
                bass.AP(wblob_own.tensor, wblob_own.offset,
                        [[416, 128], [1, 416]]), wsb[:])
            nc.gpsimd.collective_compute(
                "AllGather", OP.bypass, replica_groups=rg,
                ins=[wblob_own[:].opt()], outs=[wblob[:].opt()])

            def wload(off, kq, Dout, name):
                return cload(
                    bass.AP(wblob.tensor, wblob.offset + off,
                            [[Dout, 128], [128 * Dout, kq], [1, Dout]]),
                    [128, kq, Dout], name=name)

            wpT_s = wload(0, KQ, IN, "wpT_s")
            wl1_s = wload(131072, IN // 128, HID, "wl1_s")
            wr1_s = wload(196608, IN // 128, HID, "wr1_s")
            wm1_s = wload(262144, IN // 128, HID, "wm1_s")
            wm2_s = wload(327680, HID // 128, EMB, "wm2_s")
            wl2_s = wload(360448, HID // 128, EMB, "wl2_s")
            wr2_s = wload(393216, HID // 128, EMB, "wr2_s")
            att1_s = cload(ATT1R, [128, HID], name="att1_s")
            att2_s = cload(ATT2R, [128, EMB], name="att2_s")
            ident_s = cload(IDENT, [128, 128], name="ident_s")
            iota_s = cload(IOTA_ROWS, [128, 128], dt.float32, name="iota_s")
            a12_s = cload(A12R, [128, 2], dt.float32, name="a12_s")
            deg_s = cload(DEGT, [128, T], dt.float32, name="deg_s")
            idxe_s = cload(IDXE, [128, CKU], dt.int32, name="idxe_s")

            xr1_all = cpool.tile([128, T, IN], dt.bfloat16, name="xr1_all")
            xr2_all = cpool.tile([128, T, EMB], dt.bfloat16, name="xr2_all")

            xl1_own = dpool.tile([SH, HID], dt.bfloat16, name="xl1_own")
            xl2_own = dpool.tile([SH, EMB], dt.bfloat16, name="xl2_own")
            z_own = dpool.tile([SH, 2 * EMB], dt.bfloat16, name="z_own")
            xl1_tbl = dpool.tile([N8, HID], dt.bfloat16, name="xl1_tbl",
                                 addr_space="Shared")
            xl2_tbl = dpool.tile([N8, EMB], dt.bfloat16, name="xl2_tbl",
                                 addr_space="Shared")
            z_tbl = dpool.tile([N8, 2 * EMB], dt.bfloat16, name="z_tbl",
                               addr_space="Shared")

            # -------- helpers --------
            def ln_relu(src_t, n, D, out_bf):
                """out = relu(layer_norm(src)); scale-invariant in src."""
                sm = sstat.tile([128, 1], dt.float32, name="sm", tag="sm")
                nc.vector.tensor_reduce(sm[:n], src_t[:n, :D], axis=AX.X,
                                        op=OP.add)
                scr = sstat.tile([128, 256], dt.float32, name="scr", tag="scr")
                sq = sstat.tile([128, 1], dt.float32, name="sq", tag="sq")
                nc.scalar.activation(scr[:n, :D], src_t[:n, :D], AF.Square,
                                     accum_out=sq[:n])
                msq = sstat.tile([128, 1], dt.float32, name="msq", tag="msq")
                nc.vector.scalar_tensor_tensor(out=msq[:n], in0=sm[:n],
                                               scalar=1.0 / (D * D),
                                               in1=sm[:n], op0=OP.mult,
                                               op1=OP.mult)
                var = sstat.tile([128, 1], dt.float32, name="var", tag="var")
                nc.vector.scalar_tensor_tensor(out=var[:n], in0=sq[:n],
                                               scalar=1.0 / D, in1=msq[:n],
                                               op0=OP.mult, op1=OP.subtract)
                veps = sstat.tile([128, 1], dt.float32, name="veps", tag="veps")
                nc.vector.tensor_scalar(out=veps[:n], in0=var[:n],
                                        scalar1=EPS_LN, scalar2=None,
                                        op0=OP.add)
                rinv = sstat.tile([128, 1], dt.float32, name="rinv", tag="rinv")
                nc.vector.reciprocal(rinv[:n], veps[:n])
                rstd = sstat.tile([128, 1], dt.float32, name="rstd", tag="rstd")
                nc.scalar.activation(rstd[:n], rinv[:n], AF.Sqrt)
                nb = sstat.tile([128, 1], dt.float32, name="nb", tag="nb")
                nc.vector.scalar_tensor_tensor(out=nb[:n], in0=sm[:n],
                                               scalar=-1.0 / D, in1=rstd[:n],
                                               op0=OP.mult, op1=OP.mult)
                nc.scalar.activation(out_bf[:n, :D], src_t[:n, :D], AF.Relu,
                                     bias=nb[:n], scale=rstd[:n])

            def transpose_to(pool, src_bf, n, D, name):
                out = pool.tile([128, D // 128, 128], dt.bfloat16, name=name,
                                tag=name, padded_shape=[128, 2, 128])
                for b in range(D // 128):
                    tp = dps.tile([128, 128], dt.bfloat16, name=name + "_ps",
                                  tag="tp", space="PSUM", bufs=1)
                    nc.tensor.transpose(tp[:, :n],
                                        src_bf[:n, 128 * b:128 * (b + 1)],
                                        ident_s[:n, :n])
                    nc.scalar.copy(out[:, b, :n], tp[:, :n])
                return out

            def proj(inT, n, wT, Dout, name, kchunks):
                ps_t = dps.tile([128, 256], dt.float32, name=name + "_ps",
                                tag="proj", space="PSUM", bufs=1)
                for q in range(kchunks):
                    nc.tensor.matmul(out=ps_t[:n, :Dout], lhsT=inT[:, q, :n],
                                     rhs=wT[:, q, :], start=(q == 0),
                                     stop=(q == kchunks - 1),
                                     skip_group_check=True)
                return ps_t

            # ================= dense phase =================
            with tc.tile_pool(name="dsb", bufs=2) as dsb:
                for t in range(T):
                    n = rows(t)
                    xt = dsb.tile([128, KQ, 128], dt.float8e4, name="xt")
                    nc.sync.dma_start(
                        xt[:, :, :n],
                        xT.rearrange("(q p) m -> p q m", p=128)[:, :, 128 * t:128 * t + n])
                    xp_ps = proj(xt, n, wpT_s, IN, "xp", KQ)
                    xp = dsb.tile([128, IN], dt.bfloat16, name="xp")
                    ln_relu(xp_ps, n, IN, xp)
                    xpT = transpose_to(dsb, xp, n, IN, "xpT")

                    xl1_ps = proj(xpT, n, wl1_s, HID, "xl1", IN // 128)
                    xl1_bf = dsb.tile([128, HID], dt.bfloat16, name="xl1_bf")
                    nc.scalar.copy(xl1_bf[:n, :], xl1_ps[:n, :HID])
                    nc.sync.dma_start(xl1_own[128 * t:128 * t + n, :],
                                      xl1_bf[:n, :])

                    xr1_ps = proj(xpT, n, wr1_s, HID, "xr1", IN // 128)
                    nc.vector.tensor_copy(xr1_all[:n, t, :], xr1_ps[:n, :HID])

                    m1_ps = proj(xpT, n, wm1_s, HID, "m1", IN // 128)
                    m1 = dsb.tile([128, HID], dt.bfloat16, name="m1")
                    ln_relu(m1_ps, n, HID, m1)
                    m1T = transpose_to(dsb, m1, n, HID, "m1T")
                    zf_ps = proj(m1T, n, wm2_s, EMB, "zf", HID // 128)
                    zf_bf = dsb.tile([128, EMB], dt.bfloat16, name="zf_bf")
                    nc.vector.tensor_copy(zf_bf[:n, :], zf_ps[:n, :EMB])
                    nc.sync.dma_start(z_own[128 * t:128 * t + n, EMB:],
                                      zf_bf[:n, :])

            if stage >= 2:
                nc.gpsimd.collective_compute(
                    "AllGather", OP.bypass, replica_groups=rg,
                    ins=[xl1_own[:].opt()], outs=[xl1_tbl[:].opt()])

            # pad mask for ALL tiles in one op: mpad_all[p, t*KMAX+k] = (k >= deg[p,t])
            mpad_all = cpool.tile([128, T * KMAX], dt.float32, name="mpad_all")
            nc.vector.tensor_tensor(
                out=mpad_all[:],
                in0=bass.AP(iota_s.tensor, iota_s.offset,
                            [list(iota_s.ap[0]), [0, T], [1, KMAX]]),
                in1=bass.AP(deg_s.tensor, deg_s.offset,
                            [list(deg_s.ap[0]), [1, T], [0, KMAX]]),
                op=OP.is_ge)

            # ================= edge phase (CSR wide ops) =================
            def edge_tile(pools, t, xr_all, tbl, D, H, att_s, out_cb, suf):
                esb = pools["esb"]
                n = rows(t)
                Kt = int(K_t[t])
                c0 = int(OFF[t])
                DH = D // H
                psem = sems[suf]

                xg = esb.tile([128, Kt * D], dt.bfloat16, name=f"xg{suf}",
                              tag=f"xg{suf}", padded_shape=[128, KMAX * D])
                for k in range(Kt):
                    nc.gpsimd.indirect_dma_start(
                        out=xg[:, k * D:(k + 1) * D], out_offset=None,
                        in_=tbl[:],
                        in_offset=bass.IndirectOffsetOnAxis(
                            ap=idxe_s[:, c0 + k:c0 + k + 1], axis=0),
                    ).then_inc(psem, 16)
                gcnt[suf] += Kt
                nc.vector.tensor_copy(xg[:1, :1], xg[:1, :1])._wait_ge(
                    psem, 16 * gcnt[suf])

                # e = lrelu(xg + xr[dst]) ; score = <e, att> per head
                e_t = esb.tile([128, Kt * D], dt.bfloat16, name=f"e{suf}",
                               tag=f"e{suf}", padded_shape=[128, KMAX * D])
                xr_b = bass.AP(xr_all.tensor, xr_all.offset + t * D,
                               [list(xr_all.ap[0]), [0, Kt], [1, D]])
                nc.vector.tensor_tensor(out=e_t[:, :Kt * D],
                                        in0=xg[:, :Kt * D], in1=xr_b,
                                        op=OP.add)
                e2_t = esb.tile([128, Kt * D], dt.bfloat16, name=f"e2{suf}",
                                tag=f"e2{suf}", padded_shape=[128, KMAX * D])
                nc.vector.scalar_tensor_tensor(
                    out=e2_t[:, :Kt * D], in0=e_t[:, :Kt * D], scalar=0.2,
                    in1=e_t[:, :Kt * D], op0=OP.mult, op1=OP.max)
                att_b = bass.AP(att_s.tensor, att_s.offset,
                                [list(att_s.ap[0]), [0, Kt], [1, D]])
                sm_t = esb.tile([128, Kt * D], dt.bfloat16, name=f"smt{suf}",
                                tag=f"e{suf}", padded_shape=[128, KMAX * D])
                nc.vector.tensor_tensor(out=sm_t[:, :Kt * D],
                                        in0=e2_t[:, :Kt * D], in1=att_b,
                                        op=OP.mult)
                sc = esb.tile([128, Kt * H], dt.float32, name=f"sc{suf}",
                              tag=f"sc{suf}", padded_shape=[128, KMAX * H])
                nc.vector.tensor_reduce(
                    out=sc[:, :Kt * H],
                    in_=bass.AP(sm_t.tensor, sm_t.offset,
                                [list(sm_t.ap[0]), [DH, Kt * H], [1, DH]]),
                    axis=AX.X, op=OP.add)
                # mask pad slots: score += -100 * mpad   (broadcast over heads)
                mpad_b = bass.AP(mpad_all.tensor, mpad_all.offset + t * KMAX,
                                 [list(mpad_all.ap[0]), [1, Kt], [0, H]])
                nc.vector.scalar_tensor_tensor(
                    out=sc[:, :Kt * H], in0=mpad_b, scalar=-100.0,
                    in1=sc[:, :Kt * H], op0=OP.mult, op1=OP.add)
                ex = esb.tile([128, Kt * H], dt.float32, name=f"ex{suf}",
                              tag=f"ex{suf}", padded_shape=[128, KMAX * H])
                nc.scalar.activation(ex[:, :Kt * H], sc[:, :Kt * H], AF.Exp)
                den = sstat.tile([128, 8], dt.float32, name=f"den{suf}",
                                 tag=f"den{suf}")
                nc.vector.tensor_reduce(
                    out=den[:, :H],
                    in_=bass.AP(ex.tensor, ex.offset,
                                [list(ex.ap[0]), [1, H], [H, Kt]]),
                    axis=AX.X, op=OP.add)
                # no +eps: the self-loop term keeps den >= exp(score_self) > 0
                rec = sstat.tile([128, 8], dt.float32, name=f"rec{suf}",
                                 tag=f"rec{suf}")
                nc.vector.reciprocal(rec[:, :H], den[:, :H])
                alp = esb.tile([128, Kt * H], dt.bfloat16, name=f"al{suf}",
                               tag=f"al{suf}", padded_shape=[128, KMAX * H])
                rec_b = bass.AP(rec.tensor, rec.offset,
                                [list(rec.ap[0]), [0, Kt], [1, H]])
                nc.vector.tensor_tensor(out=alp[:, :Kt * H],
                                        in0=ex[:, :Kt * H], in1=rec_b,
                                        op=OP.mult)
                # w = xg * alpha ; out = sum_k w
                w_t = esb.tile([128, Kt * D], dt.bfloat16, name=f"w{suf}",
                               tag=f"e2{suf}", padded_shape=[128, KMAX * D])
                alp_b = bass.AP(alp.tensor, alp.offset,
                                [list(alp.ap[0]), [H, Kt], [1, H], [0, DH]])
                nc.vector.tensor_tensor(out=w_t[:, :Kt * D],
                                        in0=xg[:, :Kt * D], in1=alp_b,
                                        op=OP.mult)
                outf = esb.tile([128, D], dt.float32, name=f"o{suf}",
                                tag=f"o{suf}")
                nc.vector.tensor_reduce(
                    out=outf[:, :D],
                    in_=bass.AP(w_t.tensor, w_t.offset,
                                [list(w_t.ap[0]), [1, D], [D, Kt]]),
                    axis=AX.X, op=OP.add)
                out_cb(pools, outf, n, t)

            def l1_out(pools, outf, n, t):
                esb = pools["esb"]
                h_bf = esb.tile([128, HID], dt.bfloat16, name="h_bf",
                                tag="h_bf")
                ln_relu(outf, n, HID, h_bf)
                hT = transpose_to(esb, h_bf, n, HID, "hT")
                xl2_ps = proj(hT, n, wl2_s, EMB, "xl2", HID // 128)
                xl2_bf = esb.tile([128, EMB], dt.bfloat16, name="xl2_bf",
                                  tag="xl2_bf")
                nc.scalar.copy(xl2_bf[:n, :], xl2_ps[:n, :EMB])
                nc.sync.dma_start(xl2_own[128 * t:128 * t + n, :],
                                  xl2_bf[:n, :])
                xr2_ps = proj(hT, n, wr2_s, EMB, "xr2", HID // 128)
                nc.vector.tensor_copy(xr2_all[:n, t, :], xr2_ps[:n, :EMB])

            def l2_out(pools, outf, n, t):
                esb = pools["esb"]
                zg = esb.tile([128, EMB], dt.bfloat16, name="zg", tag="zg")
                nc.vector.tensor_copy(zg[:n, :], outf[:n, :EMB])
                nc.sync.dma_start(z_own[128 * t:128 * t + n, :EMB], zg[:n, :])

            if stage >= 3:
                with tc.tile_pool(name="esb_a", bufs=1) as esb_a:
                    pools = {"esb": esb_a}
                    for t in range(T):
                        edge_tile(pools, t, xr1_all, xl1_tbl, HID, 4, att1_s,
                                  l1_out, "a")

            if stage >= 4:
                nc.gpsimd.collective_compute(
                    "AllGather", OP.bypass, replica_groups=rg,
                    ins=[xl2_own[:].opt()], outs=[xl2_tbl[:].opt()])

                with tc.tile_pool(name="esb_b", bufs=1) as esb_b:
                    pools = {"esb": esb_b}
                    for t in range(T):
                        edge_tile(pools, t, xr2_all, xl2_tbl, EMB, 1, att2_s,
                                  l2_out, "b")

                nc.gpsimd.collective_compute(
                    "AllGather", OP.bypass, replica_groups=rg,
                    ins=[z_own[:].opt()], outs=[z_tbl[:].opt()])

            # ================= decode =================
            D2 = 2 * EMB
            NCOL = cfg.PPC // 128          # 256
            CC = 32                        # columns per chunk
            res_sb = cpool.tile([128, NCOL], dt.float32, name="res_sb")
            if stage < 5:
                nc.vector.memset(res_sb[:], 0.0)
            with tc.tile_pool(name="dec", bufs=1) as dec:
                if stage >= 5:
                    pi_t = cpool.tile([128, NCOL], dt.int32, name="pi")
                    nc.sync.dma_start(pi_t[:], PS32)
                    pj_t = cpool.tile([128, NCOL], dt.int32, name="pj")
                    nc.sync.dma_start(pj_t[:], PD32)
                for ch in range(NCOL // CC if stage >= 5 else 0):
                    o = ch * CC
                    za = dec.tile([128, CC * D2], dt.bfloat16, name="za",
                                  tag="za")
                    zb = dec.tile([128, CC * D2], dt.bfloat16, name="zb",
                                  tag="zb")
                    for j in range(CC):
                        nc.gpsimd.indirect_dma_start(
                            out=za[:, j * D2:(j + 1) * D2], out_offset=None,
                            in_=z_tbl[:],
                            in_offset=bass.IndirectOffsetOnAxis(
                                ap=pi_t[:, o + j:o + j + 1], axis=0),
                        ).then_inc(sems["d"], 16)
                        nc.gpsimd.indirect_dma_start(
                            out=zb[:, j * D2:(j + 1) * D2], out_offset=None,
                            in_=z_tbl[:],
                            in_offset=bass.IndirectOffsetOnAxis(
                                ap=pj_t[:, o + j:o + j + 1], axis=0),
                        ).then_inc(sems["d"], 16)
                    gcnt["d"] += 2 * CC
                    nc.vector.tensor_copy(za[:1, :1], za[:1, :1])._wait_ge(
                        sems["d"], 16 * gcnt["d"])
                    nc.vector.tensor_copy(zb[:1, :1], zb[:1, :1])._wait_ge(
                        sems["d"], 16 * gcnt["d"])

                    prod = dec.tile([128, CC * D2], dt.float32, name="prod",
                                    tag="prod")
                    view = lambda t_: bass.AP(
                        t_.tensor, t_.offset,
                        [list(t_.ap[0]), [EMB, CC * 2], [1, EMB]])
                    dots = dec.tile([128, CC * 2], dt.float32, name="dots",
                                    tag="dots")
                    nc.vector.tensor_tensor(out=prod[:], in0=za[:], in1=zb[:],
                                            op=OP.mult)
                    nc.vector.tensor_reduce(out=dots[:], in_=view(prod),
                                            axis=AX.X, op=OP.add)
                    sqa = dec.tile([128, CC * 2], dt.float32, name="sqa",
                                   tag="sqa")
                    nc.vector.tensor_tensor(out=prod[:], in0=za[:], in1=za[:],
                                            op=OP.mult)
                    nc.vector.tensor_reduce(out=sqa[:], in_=view(prod),
                                            axis=AX.X, op=OP.add)
                    sqb = dec.tile([128, CC * 2], dt.float32, name="sqb",
                                   tag="sqb")
                    nc.vector.tensor_tensor(out=prod[:], in0=zb[:], in1=zb[:],
                                            op=OP.mult)
                    nc.vector.tensor_reduce(out=sqb[:], in_=view(prod),
                                            axis=AX.X, op=OP.add)
                    nn_ = dec.tile([128, CC * 2], dt.float32, name="nn_",
                                   tag="nn_")
                    nc.vector.tensor_tensor(out=nn_[:], in0=sqa[:],
                                            in1=sqb[:], op=OP.mult)
                    rin = dec.tile([128, CC * 2], dt.float32, name="rin",
                                   tag="rin")
                    nc.vector.reciprocal(rin[:], nn_[:])
                    rsq = dec.tile([128, CC * 2], dt.float32, name="rsq",
                                   tag="rsq")
                    nc.scalar.activation(rsq[:], rin[:], AF.Sqrt)
                    cosv = dec.tile([128, CC * 2], dt.float32, name="cosv",
                                    tag="cosv")
                    nc.vector.tensor_tensor(out=cosv[:], in0=dots[:],
                                            in1=rsq[:], op=OP.mult)
                    wz = dec.tile([128, CC * 2], dt.float32, name="wz",
                                  tag="wz")
                    a12b = bass.AP(a12_s.tensor, a12_s.offset,
                                   [list(a12_s.ap[0]), [0, CC], [1, 2]])
                    nc.vector.tensor_tensor(out=wz[:], in0=cosv[:], in1=a12b,
                                            op=OP.mult)
                    nc.vector.tensor_reduce(
                        out=res_sb[:, o:o + CC],
                        in_=bass.AP(wz.tensor, wz.offset,
                                    [list(wz.ap[0]), [2, CC], [1, 2]]),
                        axis=AX.X, op=OP.add)

            nc.sync.dma_start(res_out.rearrange("(a b) -> b a", b=128),
                              res_sb[:])

    nc.compile()
    return nc


# ---------------------------------------------------------------------------
# entry point
# ---------------------------------------------------------------------------

def make_in_maps(plan, W, cfg):
    in_maps = []
    CKU = None
    for c in range(cfg.NC):
        m = {"xT": plan.xT[c], "DEGT": plan.DEGT[c],
             "PS32": plan.PS[c], "PD32": plan.PD[c],
             "WBLOB": W["WBLOB"][c]}
        for k in ("ATT1R", "ATT2R", "IDENT", "IOTA_ROWS", "A12R"):
            m[k] = W[k]
        in_maps.append(m)
    return in_maps


def finish_in_maps(in_maps, plan, cfg, nc):
    """Re-pack IDXE per core to the unified per-tile offsets of the program."""
    K_t = np.stack([plan.K_t[c] for c in range(cfg.NC)]).max(axis=0)
    OFF = np.concatenate([[0], np.cumsum(K_t)]).astype(np.int64)
    CKU = int(OFF[-1])
    for c in range(cfg.NC):
        idxe = np.zeros((128, CKU), dtype=np.int32)
        for t in range(cfg.T):
            kc = int(plan.K_t[c][t])
            oc = int(plan.OFF_t[c][t])
            idxe[:, int(OFF[t]):int(OFF[t]) + kc] = \
                plan.IDXE[c][:, oc:oc + kc]
        in_maps[c]["IDXE"] = idxe
    return in_maps


def kernel(**inputs):
    cfg = CFG
    plan = host_prep(inputs["x"], inputs["edge_index"],
                     inputs["edge_pairs"], cfg)
    W = prep_weights(inputs, cfg)
    nc = build_program(plan, cfg)
    from concourse.bass_utils import run_bass_kernel_spmd
    in_maps = finish_in_maps(make_in_maps(plan, W, cfg), plan, cfg, nc)
    res = run_bass_kernel_spmd(nc, in_maps, core_ids=list(range(cfg.NC)))
    out = np.concatenate([np.asarray(res.results[c]["res"])
                          for c in range(cfg.NC)])
    return out.astype(np.float32)


# revision 17
# speedup vs baseline: 1.0052x; 1.0052x over previous
"""Trainium2 Bass kernel for nn_DualSignalLinkPredictorC (2-layer GATv2 + MLP
link predictor), distributed over 8 NeuronCores.

v2: degree-sorted CSR layout. The backend executes ~1 instruction per 45us
regardless of width, so the design minimizes instruction count:
  - dst nodes on partitions, neighbors padded along the free dim (K_t = max
    in-degree per 128-node tile). Per-core nodes are permuted by descending
    degree so K_t tracks the local mean (~18) instead of the global max.
  - per-edge attention, segment softmax, and aggregation are wide DVE ops
    over [128, K_t*D] tiles; no one-hot matmuls, no PE transposes in the
    edge phase. Gathers are one indirect DMA per neighbor column.
  - single AllGather per table (no chunking; int32 row indices).
  - x is uploaded in fp8 (e4m3) to halve the dominant input transfer.
"""

import numpy as np
import ml_dtypes

BF16 = ml_dtypes.bfloat16
FP8 = ml_dtypes.float8_e4m3fn


class Cfg:
    def __init__(self, N=100000, E=1600000, NPAIRS=262144, NC=8,
                 RAW=512, IN=256, HID=256, EMB=128):
        self.N, self.E, self.NPAIRS, self.NC = N, E, NPAIRS, NC
        self.RAW, self.IN, self.HID, self.EMB = RAW, IN, HID, EMB
        assert N % NC == 0
        self.SH = N // NC
        self.T = (self.SH + 127) // 128
        self.PPC = NPAIRS // NC
        assert self.PPC % 128 == 0


CFG = Cfg()


class Plan:
    """Degree-sorted CSR neighbor plan + decode indices, per core."""

    def __init__(self, cfg, x, edge_index, edge_pairs):
        NC, SH, T = cfg.NC, cfg.SH, cfg.T
        ei = np.asarray(edge_index, dtype=np.int64)
        ep = np.asarray(edge_pairs, dtype=np.int64)
        loops = np.arange(cfg.N, dtype=np.int64)
        src = np.concatenate([ei[0], loops])
        dst = np.concatenate([ei[1], loops])

        deg = np.bincount(dst, minlength=cfg.N)
        # per-core descending-degree permutation; global_row maps node id ->
        # row in the AllGathered tables (core-major, rank within core).
        self.perm = []           # per core: rank -> local node
        global_row = np.empty(cfg.N, dtype=np.int64)
        for c in range(NC):
            dloc = deg[c * SH:(c + 1) * SH]
            p = np.argsort(-dloc, kind="stable")
            self.perm.append(p)
            rank = np.empty(SH, dtype=np.int64)
            rank[p] = np.arange(SH)
            global_row[c * SH:(c + 1) * SH] = c * SH + rank
        self.global_row = global_row

        srcrow = global_row[src]
        core_of = dst // SH
        rank_of = global_row[dst] - core_of * SH

        self.IDXE, self.DEGT, self.K_t, self.OFF_t = [], [], [], []
        for c in range(NC):
            m = core_of == c
            r = rank_of[m]
            v = srcrow[m]
            order = np.argsort(r, kind="stable")
            r, v = r[order], v[order]
            dsorted = np.zeros(SH, dtype=np.int64)
            dsorted[:SH] = np.bincount(r, minlength=SH)
            starts = np.concatenate([[0], np.cumsum(dsorted)])
            K_t = np.zeros(T, dtype=np.int64)
            for t in range(T):
                K_t[t] = dsorted[128 * t:128 * (t + 1)].max()
            OFF = np.concatenate([[0], np.cumsum(K_t)]).astype(np.int64)
            CK = int(OFF[-1])
            idxe = np.zeros((128, CK), dtype=np.int32)
            slot_in_dst = np.arange(len(r)) - starts[r]
            tile_of = r >> 7
            p_of = r & 127
            col = OFF[tile_of] + slot_in_dst
            idxe[p_of, col] = v
            degt = np.zeros((128, T), dtype=np.float32)
            dpad = np.concatenate([dsorted, np.zeros(T * 128 - SH, np.int64)])
            degt[:, :] = dpad.reshape(T, 128).T
            self.IDXE.append(np.ascontiguousarray(idxe))
            self.DEGT.append(degt)
            self.K_t.append(K_t)
            self.OFF_t.append(OFF)
        self.CKmax = max(int(o[-1]) for o in self.OFF_t)
        self.KMAX = max(int(k.max()) for k in self.K_t)
        # pad every core's IDXE to CKmax columns so shapes match SPMD
        for c in range(NC):
            ck = self.IDXE[c].shape[1]
            if ck < self.CKmax:
                self.IDXE[c] = np.ascontiguousarray(np.pad(
                    self.IDXE[c], ((0, 0), (0, self.CKmax - ck))))

        # decode: pair i of core c -> (p=i%128, col=i//128)
        self.PS, self.PD = [], []
        pr = global_row[ep[:, 0]].reshape(NC, cfg.PPC)
        qr = global_row[ep[:, 1]].reshape(NC, cfg.PPC)
        for c in range(NC):
            ps = pr[c].reshape(cfg.PPC // 128, 128).T.astype(np.int32)
            pd = qr[c].reshape(cfg.PPC // 128, 128).T.astype(np.int32)
            self.PS.append(np.ascontiguousarray(ps))
            self.PD.append(np.ascontiguousarray(pd))

        x = np.nan_to_num(np.asarray(x, dtype=np.float32), nan=0.0,
                          posinf=0.0, neginf=0.0)
        self.xT = []
        for c in range(NC):
            xs = x[c * SH:(c + 1) * SH][self.perm[c]]
            self.xT.append(np.ascontiguousarray(xs.T.astype(FP8)))


def host_prep(x, edge_index, edge_pairs, cfg):
    return Plan(cfg, x, edge_index, edge_pairs)


def prep_weights(inp, cfg):
    f = lambda a: np.asarray(a, np.float32)
    W = {}
    # projection weights ship sharded (1/8 per core) and are reassembled on
    # device by one AllGather; blob order must match build_program's offsets.
    blob = np.concatenate([
        f(inp["Wp"]).T.astype(BF16).ravel()] + [
        f(inp[k]).T.astype(BF16).ravel()
        for k in ("Wl1", "Wr1", "Wm1", "Wm2", "Wl2", "Wr2")])
    assert blob.size == 425984
    W["WBLOB"] = [np.ascontiguousarray(
        blob[c * 53248:(c + 1) * 53248].reshape(208, 256))
        for c in range(cfg.NC)]
    att1 = np.broadcast_to(f(inp["att1"]).reshape(1, -1),
                           (128, cfg.HID)).astype(BF16)
    att2 = np.broadcast_to(f(inp["att2"]).reshape(1, -1),
                           (128, cfg.EMB)).astype(BF16)
    ident = np.eye(128, dtype=np.float32).astype(BF16)
    W["BF16C"] = np.ascontiguousarray(
        np.concatenate([att1, att2, ident], axis=1))      # [128, 512]
    iota = np.broadcast_to(np.arange(128, dtype=np.float32), (128, 128))
    alpha = 1.0 / (1.0 + np.exp(-float(f(inp["logit_alpha"]).ravel()[0])))
    temp = float(f(inp["temperature"]))
    a12 = np.broadcast_to(
        np.array([alpha * temp, (1.0 - alpha) * temp], np.float32), (128, 2))
    W["F32C"] = [iota, a12]   # per-core DEGT is prepended in make_in_maps
    return W


# ---------------------------------------------------------------------------
# device program
# ---------------------------------------------------------------------------

def build_program(plan, cfg, stage=5):
    import contextlib
    import concourse.bass as bass
    import concourse.tile as tile
    from concourse import bacc, mybir

    dt = mybir.dt
    AF = mybir.ActivationFunctionType
    OP = mybir.AluOpType
    AX = mybir.AxisListType

    NC, SH, T = cfg.NC, cfg.SH, cfg.T
    RAW, IN, HID, EMB = cfg.RAW, cfg.IN, cfg.HID, cfg.EMB
    KQ = RAW // 128
    N8 = SH * NC
    CK = plan.CKmax
    KMAX = plan.KMAX
    K_t, OFF_t = plan.K_t[0], plan.OFF_t[0]   # identical structure per core?
    EPS_LN = 1e-5
    EPS_DEN = 1e-16

    # NOTE: K_t / OFF_t differ per core. SPMD emits ONE program, so loop
    # bounds must be core-independent: use the per-tile MAX over cores.
    K_t = np.stack([plan.K_t[c] for c in range(NC)]).max(axis=0)
    OFF = np.concatenate([[0], np.cumsum(K_t)]).astype(np.int64)
    assert OFF[-1] <= CK or True
    CKU = int(OFF[-1])

    nc = bacc.Bacc("TRN2", target_bir_lowering=False, debug=False,
                   num_devices=NC)

    din = lambda name, shape, d: nc.dram_tensor(name, shape, d,
                                                kind="ExternalInput").ap()
    NCOL0 = cfg.PPC // 128
    xT = din("xT", [RAW, SH], dt.float8e4)
    IDXE = din("IDXE", [128, CKU], dt.int32)
    PSPD = din("PSPD", [128, 2 * NCOL0], dt.int32)
    WBLOB = din("WBLOB", [208, 256], dt.bfloat16)
    BF16C = din("BF16C", [128, HID + EMB + 128], dt.bfloat16)
    F32C = din("F32C", [128, T + 130], dt.float32)

    res_out = nc.dram_tensor("res", [cfg.PPC], dt.float32,
                             kind="ExternalOutput").ap()

    rg = [list(range(NC))]
    sems = {ph: nc.alloc_semaphore(f"gsem_{ph}") for ph in ("a", "b", "d")}
    gcnt = {ph: 0 for ph in ("a", "b", "d")}

    def rows(t):
        return min(128, SH - 128 * t)

    with tile.TileContext(nc) as tc:
        ctx = contextlib.ExitStack()
        with ctx:
            cpool = ctx.enter_context(tc.tile_pool(name="consts", bufs=1))
            dpool = ctx.enter_context(tc.tile_pool(name="dram", bufs=1,
                                                   space="DRAM"))
            sstat = ctx.enter_context(tc.tile_pool(name="sstat", bufs=2))
            dps = ctx.enter_context(tc.tile_pool(name="dps", bufs=2,
                                                 space="PSUM"))

            def cload(ap, shape, d=dt.bfloat16, name=None):
                t_ = cpool.tile(shape, d, name=name)
                nc.sync.dma_start(t_[:], ap)
                return t_

            # reassemble the sharded weight blob: upload -> own DRAM slice ->
            # AllGather -> per-weight strided loads into SBUF.
            wblob_own = dpool.tile([208, 256], dt.bfloat16, name="wblob_own")
            wblob = dpool.tile([1664, 256], dt.bfloat16, name="wblob",
                               addr_space="Shared")
            wsb = cpool.tile([128, 416], dt.bfloat16, name="wsb")
            nc.sync.dma_start(
                wsb[:], bass.AP(WBLOB.tensor, 0, [[416, 128], [1, 416]]))
            nc.sync.dma_start(
                bass.AP(wblob_own.tensor, wblob_own.offset,
                        [[416, 128], [1, 416]]), wsb[:])
            nc.gpsimd.collective_compute(
                "AllGather", OP.bypass, replica_groups=rg,
                ins=[wblob_own[:].opt()], outs=[wblob[:].opt()])

            def wload(off, kq, Dout, name):
                return cload(
                    bass.AP(wblob.tensor, wblob.offset + off,
                            [[Dout, 128], [128 * Dout, kq], [1, Dout]]),
                    [128, kq, Dout], name=name)

            wpT_s = wload(0, KQ, IN, "wpT_s")
            wl1_s = wload(131072, IN // 128, HID, "wl1_s")
            wr1_s = wload(196608, IN // 128, HID, "wr1_s")
            wm1_s = wload(262144, IN // 128, HID, "wm1_s")
            wm2_s = wload(327680, HID // 128, EMB, "wm2_s")
            wl2_s = wload(360448, HID // 128, EMB, "wl2_s")
            wr2_s = wload(393216, HID // 128, EMB, "wr2_s")
            att1_s = cload(BF16C[:, :HID], [128, HID], name="att1_s")
            att2_s = cload(BF16C[:, HID:HID + EMB], [128, EMB], name="att2_s")
            ident_s = cload(BF16C[:, HID + EMB:], [128, 128], name="ident_s")
            deg_s = cload(F32C[:, :T], [128, T], dt.float32, name="deg_s")
            iota_s = cload(F32C[:, T:T + 128], [128, 128], dt.float32,
                           name="iota_s")
            a12_s = cload(F32C[:, T + 128:], [128, 2], dt.float32,
                          name="a12_s")
            idxe_s = cload(IDXE, [128, CKU], dt.int32, name="idxe_s")

            xr1_all = cpool.tile([128, T, IN], dt.bfloat16, name="xr1_all")
            xr2_all = cpool.tile([128, T, EMB], dt.bfloat16, name="xr2_all")

            xl1_own = dpool.tile([SH, HID], dt.bfloat16, name="xl1_own")
            xl2_own = dpool.tile([SH, EMB], dt.bfloat16, name="xl2_own")
            z_own = dpool.tile([SH, 2 * EMB], dt.bfloat16, name="z_own")
            xl1_tbl = dpool.tile([N8, HID], dt.bfloat16, name="xl1_tbl",
                                 addr_space="Shared")
            xl2_tbl = dpool.tile([N8, EMB], dt.bfloat16, name="xl2_tbl",
                                 addr_space="Shared")
            z_tbl = dpool.tile([N8, 2 * EMB], dt.bfloat16, name="z_tbl",
                               addr_space="Shared")

            # -------- helpers --------
            def ln_relu(src_t, n, D, out_bf):
                """out = relu(layer_norm(src)); scale-invariant in src."""
                sm = sstat.tile([128, 1], dt.float32, name="sm", tag="sm")
                nc.vector.tensor_reduce(sm[:n], src_t[:n, :D], axis=AX.X,
                                        op=OP.add)
                scr = sstat.tile([128, 256], dt.float32, name="scr", tag="scr")
                sq = sstat.tile([128, 1], dt.float32, name="sq", tag="sq")
                nc.scalar.activation(scr[:n, :D], src_t[:n, :D], AF.Square,
                                     accum_out=sq[:n])
                msq = sstat.tile([128, 1], dt.float32, name="msq", tag="msq")
                nc.vector.scalar_tensor_tensor(out=msq[:n], in0=sm[:n],
                                               scalar=1.0 / (D * D),
                                               in1=sm[:n], op0=OP.mult,
                                               op1=OP.mult)
                var = sstat.tile([128, 1], dt.float32, name="var", tag="var")
                nc.vector.scalar_tensor_tensor(out=var[:n], in0=sq[:n],
                                               scalar=1.0 / D, in1=msq[:n],
                                               op0=OP.mult, op1=OP.subtract)
                veps = sstat.tile([128, 1], dt.float32, name="veps", tag="veps")
                nc.vector.tensor_scalar(out=veps[:n], in0=var[:n],
                                        scalar1=EPS_LN, scalar2=None,
                                        op0=OP.add)
                rinv = sstat.tile([128, 1], dt.float32, name="rinv", tag="rinv")
                nc.vector.reciprocal(rinv[:n], veps[:n])
                rstd = sstat.tile([128, 1], dt.float32, name="rstd", tag="rstd")
                nc.scalar.activation(rstd[:n], rinv[:n], AF.Sqrt)
                nb = sstat.tile([128, 1], dt.float32, name="nb", tag="nb")
                nc.vector.scalar_tensor_tensor(out=nb[:n], in0=sm[:n],
                                               scalar=-1.0 / D, in1=rstd[:n],
                                               op0=OP.mult, op1=OP.mult)
                nc.scalar.activation(out_bf[:n, :D], src_t[:n, :D], AF.Relu,
                                     bias=nb[:n], scale=rstd[:n])

            def transpose_to(pool, src_bf, n, D, name):
                out = pool.tile([128, D // 128, 128], dt.bfloat16, name=name,
                                tag=name, padded_shape=[128, 2, 128])
                for b in range(D // 128):
                    tp = dps.tile([128, 128], dt.bfloat16, name=name + "_ps",
                                  tag="tp", space="PSUM", bufs=1)
                    nc.tensor.transpose(tp[:, :n],
                                        src_bf[:n, 128 * b:128 * (b + 1)],
                                        ident_s[:n, :n])
                    nc.scalar.copy(out[:, b, :n], tp[:, :n])
                return out

            def proj(inT, n, wT, Dout, name, kchunks):
                ps_t = dps.tile([128, 256], dt.float32, name=name + "_ps",
                                tag="proj", space="PSUM", bufs=1)
                for q in range(kchunks):
                    nc.tensor.matmul(out=ps_t[:n, :Dout], lhsT=inT[:, q, :n],
                                     rhs=wT[:, q, :], start=(q == 0),
                                     stop=(q == kchunks - 1),
                                     skip_group_check=True)
                return ps_t

            # ================= dense phase =================
            with tc.tile_pool(name="dsb", bufs=2) as dsb:
                for t in range(T):
                    n = rows(t)
                    xt = dsb.tile([128, KQ, 128], dt.float8e4, name="xt")
                    nc.sync.dma_start(
                        xt[:, :, :n],
                        xT.rearrange("(q p) m -> p q m", p=128)[:, :, 128 * t:128 * t + n])
                    xp_ps = proj(xt, n, wpT_s, IN, "xp", KQ)
                    xp = dsb.tile([128, IN], dt.bfloat16, name="xp")
                    ln_relu(xp_ps, n, IN, xp)
                    xpT = transpose_to(dsb, xp, n, IN, "xpT")

                    xl1_ps = proj(xpT, n, wl1_s, HID, "xl1", IN // 128)
                    xl1_bf = dsb.tile([128, HID], dt.bfloat16, name="xl1_bf")
                    nc.scalar.copy(xl1_bf[:n, :], xl1_ps[:n, :HID])
                    nc.sync.dma_start(xl1_own[128 * t:128 * t + n, :],
                                      xl1_bf[:n, :])

                    xr1_ps = proj(xpT, n, wr1_s, HID, "xr1", IN // 128)
                    nc.vector.tensor_copy(xr1_all[:n, t, :], xr1_ps[:n, :HID])

                    m1_ps = proj(xpT, n, wm1_s, HID, "m1", IN // 128)
                    m1 = dsb.tile([128, HID], dt.bfloat16, name="m1")
                    ln_relu(m1_ps, n, HID, m1)
                    m1T = transpose_to(dsb, m1, n, HID, "m1T")
                    zf_ps = proj(m1T, n, wm2_s, EMB, "zf", HID // 128)
                    zf_bf = dsb.tile([128, EMB], dt.bfloat16, name="zf_bf")
                    nc.vector.tensor_copy(zf_bf[:n, :], zf_ps[:n, :EMB])
                    nc.sync.dma_start(z_own[128 * t:128 * t + n, EMB:],
                                      zf_bf[:n, :])

            if stage >= 2:
                nc.gpsimd.collective_compute(
                    "AllGather", OP.bypass, replica_groups=rg,
                    ins=[xl1_own[:].opt()], outs=[xl1_tbl[:].opt()])

            # pad mask for ALL tiles in one op: mpad_all[p, t*KMAX+k] = (k >= deg[p,t])
            mpad_all = cpool.tile([128, T * KMAX], dt.float32, name="mpad_all")
            nc.vector.tensor_tensor(
                out=mpad_all[:],
                in0=bass.AP(iota_s.tensor, iota_s.offset,
                            [list(iota_s.ap[0]), [0, T], [1, KMAX]]),
                in1=bass.AP(deg_s.tensor, deg_s.offset,
                            [list(deg_s.ap[0]), [1, T], [0, KMAX]]),
                op=OP.is_ge)

            # ================= edge phase (CSR wide ops) =================
            def edge_tile(pools, t, xr_all, tbl, D, H, att_s, out_cb, suf):
                esb = pools["esb"]
                n = rows(t)
                Kt = int(K_t[t])
                c0 = int(OFF[t])
                DH = D // H
                psem = sems[suf]

                xg = esb.tile([128, Kt * D], dt.bfloat16, name=f"xg{suf}",
                              tag=f"xg{suf}", padded_shape=[128, KMAX * D])
                for k in range(Kt):
                    nc.gpsimd.indirect_dma_start(
                        out=xg[:, k * D:(k + 1) * D], out_offset=None,
                        in_=tbl[:],
                        in_offset=bass.IndirectOffsetOnAxis(
                            ap=idxe_s[:, c0 + k:c0 + k + 1], axis=0),
                    ).then_inc(psem, 16)
                gcnt[suf] += Kt
                nc.vector.tensor_copy(xg[:1, :1], xg[:1, :1])._wait_ge(
                    psem, 16 * gcnt[suf])

                # e = lrelu(xg + xr[dst]) ; score = <e, att> per head
                e_t = esb.tile([128, Kt * D], dt.bfloat16, name=f"e{suf}",
                               tag=f"e{suf}", padded_shape=[128, KMAX * D])
                xr_b = bass.AP(xr_all.tensor, xr_all.offset + t * D,
                               [list(xr_all.ap[0]), [0, Kt], [1, D]])
                nc.vector.tensor_tensor(out=e_t[:, :Kt * D],
                                        in0=xg[:, :Kt * D], in1=xr_b,
                                        op=OP.add)
                e2_t = esb.tile([128, Kt * D], dt.bfloat16, name=f"e2{suf}",
                                tag=f"e2{suf}", padded_shape=[128, KMAX * D])
                nc.vector.scalar_tensor_tensor(
                    out=e2_t[:, :Kt * D], in0=e_t[:, :Kt * D], scalar=0.2,
                    in1=e_t[:, :Kt * D], op0=OP.mult, op1=OP.max)
                att_b = bass.AP(att_s.tensor, att_s.offset,
                                [list(att_s.ap[0]), [0, Kt], [1, D]])
                sm_t = esb.tile([128, Kt * D], dt.bfloat16, name=f"smt{suf}",
                                tag=f"e{suf}", padded_shape=[128, KMAX * D])
                nc.vector.tensor_tensor(out=sm_t[:, :Kt * D],
                                        in0=e2_t[:, :Kt * D], in1=att_b,
                                        op=OP.mult)
                sc = esb.tile([128, Kt * H], dt.float32, name=f"sc{suf}",
                              tag=f"sc{suf}", padded_shape=[128, KMAX * H])
                nc.vector.tensor_reduce(
                    out=sc[:, :Kt * H],
                    in_=bass.AP(sm_t.tensor, sm_t.offset,
                                [list(sm_t.ap[0]), [DH, Kt * H], [1, DH]]),
                    axis=AX.X, op=OP.add)
                # mask pad slots: score += -100 * mpad   (broadcast over heads)
                mpad_b = bass.AP(mpad_all.tensor, mpad_all.offset + t * KMAX,
                                 [list(mpad_all.ap[0]), [1, Kt], [0, H]])
                nc.vector.scalar_tensor_tensor(
                    out=sc[:, :Kt * H], in0=mpad_b, scalar=-100.0,
                    in1=sc[:, :Kt * H], op0=OP.mult, op1=OP.add)
                ex = esb.tile([128, Kt * H], dt.float32, name=f"ex{suf}",
                              tag=f"ex{suf}", padded_shape=[128, KMAX * H])
                nc.scalar.activation(ex[:, :Kt * H], sc[:, :Kt * H], AF.Exp)
                den = sstat.tile([128, 8], dt.float32, name=f"den{suf}",
                                 tag=f"den{suf}")
                nc.vector.tensor_reduce(
                    out=den[:, :H],
                    in_=bass.AP(ex.tensor, ex.offset,
                                [list(ex.ap[0]), [1, H], [H, Kt]]),
                    axis=AX.X, op=OP.add)
                # no +eps: the self-loop term keeps den >= exp(score_self) > 0
                rec = sstat.tile([128, 8], dt.float32, name=f"rec{suf}",
                                 tag=f"rec{suf}")
                nc.vector.reciprocal(rec[:, :H], den[:, :H])
                alp = esb.tile([128, Kt * H], dt.bfloat16, name=f"al{suf}",
                               tag=f"al{suf}", padded_shape=[128, KMAX * H])
                rec_b = bass.AP(rec.tensor, rec.offset,
                                [list(rec.ap[0]), [0, Kt], [1, H]])
                nc.vector.tensor_tensor(out=alp[:, :Kt * H],
                                        in0=ex[:, :Kt * H], in1=rec_b,
                                        op=OP.mult)
                # w = xg * alpha ; out = sum_k w
                w_t = esb.tile([128, Kt * D], dt.bfloat16, name=f"w{suf}",
                               tag=f"e2{suf}", padded_shape=[128, KMAX * D])
                alp_b = bass.AP(alp.tensor, alp.offset,
                                [list(alp.ap[0]), [H, Kt], [1, H], [0, DH]])
                nc.vector.tensor_tensor(out=w_t[:, :Kt * D],
                                        in0=xg[:, :Kt * D], in1=alp_b,
                                        op=OP.mult)
                outf = esb.tile([128, D], dt.float32, name=f"o{suf}",
                                tag=f"o{suf}")
                nc.vector.tensor_reduce(
                    out=outf[:, :D],
                    in_=bass.AP(w_t.tensor, w_t.offset,
                                [list(w_t.ap[0]), [1, D], [D, Kt]]),
                    axis=AX.X, op=OP.add)
                out_cb(pools, outf, n, t)

            def l1_out(pools, outf, n, t):
                esb = pools["esb"]
                h_bf = esb.tile([128, HID], dt.bfloat16, name="h_bf",
                                tag="h_bf")
                ln_relu(outf, n, HID, h_bf)
                hT = transpose_to(esb, h_bf, n, HID, "hT")
                xl2_ps = proj(hT, n, wl2_s, EMB, "xl2", HID // 128)
                xl2_bf = esb.tile([128, EMB], dt.bfloat16, name="xl2_bf",
                                  tag="xl2_bf")
                nc.scalar.copy(xl2_bf[:n, :], xl2_ps[:n, :EMB])
                nc.sync.dma_start(xl2_own[128 * t:128 * t + n, :],
                                  xl2_bf[:n, :])
                xr2_ps = proj(hT, n, wr2_s, EMB, "xr2", HID // 128)
                nc.vector.tensor_copy(xr2_all[:n, t, :], xr2_ps[:n, :EMB])

            def l2_out(pools, outf, n, t):
                esb = pools["esb"]
                zg = esb.tile([128, EMB], dt.bfloat16, name="zg", tag="zg")
                nc.vector.tensor_copy(zg[:n, :], outf[:n, :EMB])
                nc.sync.dma_start(z_own[128 * t:128 * t + n, :EMB], zg[:n, :])

            if stage >= 3:
                with tc.tile_pool(name="esb_a", bufs=1) as esb_a:
                    pools = {"esb": esb_a}
                    for t in range(T):
                        edge_tile(pools, t, xr1_all, xl1_tbl, HID, 4, att1_s,
                                  l1_out, "a")

            if stage >= 4:
                nc.gpsimd.collective_compute(
                    "AllGather", OP.bypass, replica_groups=rg,
                    ins=[xl2_own[:].opt()], outs=[xl2_tbl[:].opt()])

                with tc.tile_pool(name="esb_b", bufs=1) as esb_b:
                    pools = {"esb": esb_b}
                    for t in range(T):
                        edge_tile(pools, t, xr2_all, xl2_tbl, EMB, 1, att2_s,
                                  l2_out, "b")

                nc.gpsimd.collective_compute(
                    "AllGather", OP.bypass, replica_groups=rg,
                    ins=[z_own[:].opt()], outs=[z_tbl[:].opt()])

            # ================= decode =================
            D2 = 2 * EMB
            NCOL = cfg.PPC // 128          # 256
            CC = 32                        # columns per chunk
            res_sb = cpool.tile([128, NCOL], dt.float32, name="res_sb")
            if stage < 5:
                nc.vector.memset(res_sb[:], 0.0)
            with tc.tile_pool(name="dec", bufs=1) as dec:
                if stage >= 5:
                    pi_t = cpool.tile([128, NCOL], dt.int32, name="pi")
                    nc.sync.dma_start(pi_t[:], PSPD[:, :NCOL])
                    pj_t = cpool.tile([128, NCOL], dt.int32, name="pj")
                    nc.sync.dma_start(pj_t[:], PSPD[:, NCOL:])
                for ch in range(NCOL // CC if stage >= 5 else 0):
                    o = ch * CC
                    za = dec.tile([128, CC * D2], dt.bfloat16, name="za",
                                  tag="za")
                    zb = dec.tile([128, CC * D2], dt.bfloat16, name="zb",
                                  tag="zb")
                    for j in range(CC):
                        nc.gpsimd.indirect_dma_start(
                            out=za[:, j * D2:(j + 1) * D2], out_offset=None,
                            in_=z_tbl[:],
                            in_offset=bass.IndirectOffsetOnAxis(
                                ap=pi_t[:, o + j:o + j + 1], axis=0),
                        ).then_inc(sems["d"], 16)
                        nc.gpsimd.indirect_dma_start(
                            out=zb[:, j * D2:(j + 1) * D2], out_offset=None,
                            in_=z_tbl[:],
                            in_offset=bass.IndirectOffsetOnAxis(
                                ap=pj_t[:, o + j:o + j + 1], axis=0),
                        ).then_inc(sems["d"], 16)
                    gcnt["d"] += 2 * CC
                    nc.vector.tensor_copy(za[:1, :1], za[:1, :1])._wait_ge(
                        sems["d"], 16 * gcnt["d"])
                    nc.vector.tensor_copy(zb[:1, :1], zb[:1, :1])._wait_ge(
                        sems["d"], 16 * gcnt["d"])

                    prod = dec.tile([128, CC * D2], dt.float32, name="prod",
                                    tag="prod")
                    view = lambda t_: bass.AP(
                        t_.tensor, t_.offset,
                        [list(t_.ap[0]), [EMB, CC * 2], [1, EMB]])
                    dots = dec.tile([128, CC * 2], dt.float32, name="dots",
                                    tag="dots")
                    nc.vector.tensor_tensor(out=prod[:], in0=za[:], in1=zb[:],
                                            op=OP.mult)
                    nc.vector.tensor_reduce(out=dots[:], in_=view(prod),
                                            axis=AX.X, op=OP.add)
                    sqa = dec.tile([128, CC * 2], dt.float32, name="sqa",
                                   tag="sqa")
                    nc.vector.tensor_tensor(out=prod[:], in0=za[:], in1=za[:],
                                            op=OP.mult)
                    nc.vector.tensor_reduce(out=sqa[:], in_=view(prod),
                                            axis=AX.X, op=OP.add)
                    sqb = dec.tile([128, CC * 2], dt.float32, name="sqb",
                                   tag="sqb")
                    nc.vector.tensor_tensor(out=prod[:], in0=zb[:], in1=zb[:],
                                            op=OP.mult)
                    nc.vector.tensor_reduce(out=sqb[:], in_=view(prod),
                                            axis=AX.X, op=OP.add)
                    nn_ = dec.tile([128, CC * 2], dt.float32, name="nn_",
                                   tag="nn_")
                    nc.vector.tensor_tensor(out=nn_[:], in0=sqa[:],
                                            in1=sqb[:], op=OP.mult)
                    rin = dec.tile([128, CC * 2], dt.float32, name="rin",
                                   tag="rin")
                    nc.vector.reciprocal(rin[:], nn_[:])
                    rsq = dec.tile([128, CC * 2], dt.float32, name="rsq",
                                   tag="rsq")
                    nc.scalar.activation(rsq[:], rin[:], AF.Sqrt)
                    cosv = dec.tile([128, CC * 2], dt.float32, name="cosv",
                                    tag="cosv")
                    nc.vector.tensor_tensor(out=cosv[:], in0=dots[:],
                                            in1=rsq[:], op=OP.mult)
                    wz = dec.tile([128, CC * 2], dt.float32, name="wz",
                                  tag="wz")
                    a12b = bass.AP(a12_s.tensor, a12_s.offset,
                                   [list(a12_s.ap[0]), [0, CC], [1, 2]])
                    nc.vector.tensor_tensor(out=wz[:], in0=cosv[:], in1=a12b,
                                            op=OP.mult)
                    nc.vector.tensor_reduce(
                        out=res_sb[:, o:o + CC],
                        in_=bass.AP(wz.tensor, wz.offset,
                                    [list(wz.ap[0]), [2, CC], [1, 2]]),
                        axis=AX.X, op=OP.add)

            nc.sync.dma_start(res_out.rearrange("(a b) -> b a", b=128),
                              res_sb[:])

    nc.compile()
    return nc


# ---------------------------------------------------------------------------
# entry point
# ---------------------------------------------------------------------------

def make_in_maps(plan, W, cfg):
    in_maps = []
    CKU = None
    iota, a12 = W["F32C"]
    for c in range(cfg.NC):
        m = {"xT": plan.xT[c],
             "PSPD": np.ascontiguousarray(
                 np.concatenate([plan.PS[c], plan.PD[c]], axis=1)),
             "WBLOB": W["WBLOB"][c],
             "BF16C": W["BF16C"],
             "F32C": np.ascontiguousarray(np.concatenate(
                 [plan.DEGT[c], iota, a12], axis=1).astype(np.float32))}
        in_maps.append(m)
    return in_maps


def finish_in_maps(in_maps, plan, cfg, nc):
    """Re-pack IDXE per core to the unified per-tile offsets of the program."""
    K_t = np.stack([plan.K_t[c] for c in range(cfg.NC)]).max(axis=0)
    OFF = np.concatenate([[0], np.cumsum(K_t)]).astype(np.int64)
    CKU = int(OFF[-1])
    for c in range(cfg.NC):
        idxe = np.zeros((128, CKU), dtype=np.int32)
        for t in range(cfg.T):
            kc = int(plan.K_t[c][t])
            oc = int(plan.OFF_t[c][t])
            idxe[:, int(OFF[t]):int(OFF[t]) + kc] = \
                plan.IDXE[c][:, oc:oc + kc]
        in_maps[c]["IDXE"] = idxe
    return in_maps


def kernel(**inputs):
    cfg = CFG
    plan = host_prep(inputs["x"], inputs["edge_index"],
                     inputs["edge_pairs"], cfg)
    W = prep_weights(inputs, cfg)
    nc = build_program(plan, cfg)
    from concourse.bass_utils import run_bass_kernel_spmd
    in_maps = finish_in_maps(make_in_maps(plan, W, cfg), plan, cfg, nc)
    res = run_bass_kernel_spmd(nc, in_maps, core_ids=list(range(cfg.NC)))
    out = np.concatenate([np.asarray(res.results[c]["res"])
                          for c in range(cfg.NC)])
    return out.astype(np.float32)
